# revision 7
# baseline (speedup 1.0000x reference)
"""Trainium2 Bass kernel for nn_Conv_38225208934663 (gnn_message_passing).

Strategy (edge-parallel, dst-sharded):
  - Edges are sharded across 8 cores by destination-node range (4096 nodes/core),
    sorted by dst, grouped into 32 node-tiles of 128 nodes, each group's edge
    list padded to K chunks of 128 edges.  Each core computes its node range's
    output completely -- no collectives needed.
  - Per chunk of 128 edges (on-chip):
      radial MLP:  hT = silu(w1.T @ edge_attrT + b1)            (PE + ACT)
      tp weights:  T  = (hb-chunk).T @ w2perm  -> [128e, 1024]  (PE, bf16)
      per-edge TP: t[e, inst*256+o*16+u] = T * v  via 3 broadcast-AP
                   tensor_tensor ops                            (DVE, bf16)
      scatter:     npsum[128n, 2048] += B.T @ t, B one-hot of dst
                   built on GPSIMD via is_equal vs an iota tile (PE accumulate)
  - Per group of 128 nodes: u-reduce npsum (strided DVE reduce), combine paths,
    multiply by alpha/max(cnt,1), DMA out.

Algebra: msg = ALPHA*concat(p00+p11, p01+p10) with all paths expressed as
  sum_u T_path[o,u] * v[u], v in {x0*sh0, dot(x1,sh1)/sqrt3, x0*sh1_m, x1_m*sh0};
  ALPHA and 1/sqrt3 are folded into w2perm on the host.
"""
import sys
import numpy as np

sys.path.insert(0, "/opt/trn_rl_repo")

import concourse.bass as bass
import concourse.bacc as bacc
import concourse.tile as tile
from concourse import mybir
import ml_dtypes

BF16NP = ml_dtypes.bfloat16
F32 = mybir.dt.float32
BF16 = mybir.dt.bfloat16

N = 32768
E = 262144
MUL = 16
NCORES = 8
NPC = N // NCORES          # 4096 nodes per core
GROUPS = NPC // 128        # 32 node-tiles per core
SQRT3 = 1.7320508075688772
ALPHA = 1.0 / np.sqrt(2.0 * MUL).astype(np.float32)

LAST_RESULT = None         # BassKernelResults of the most recent run (for test.py)
_PROGRAM_CACHE = {}


def _mkap(t, offset_elems, dims):
    """Manual free-dim AP on a tile AP `t[:]`: keep partition dim, custom free dims."""
    ap = t[:]
    return bass.AP(tensor=ap.tensor, offset=ap.offset + offset_elems,
                   ap=[list(ap.ap[0])] + [list(d) for d in dims])


def _dram_ap(dram, offset_elems, dims):
    ap = dram
    return bass.AP(tensor=ap.tensor, offset=ap.offset + offset_elems,
                   ap=[list(d) for d in dims])


def _build_program(K, b2_nonzero):
    """Build + compile the SPMD single-core program for K chunks/group."""
    C = GROUPS * K           # chunks per core
    Ep = C * 128             # padded edges per core
    assert C % 4 == 0
    S = C // 4               # supertiles (512 edges each)

    nc = bacc.Bacc("TRN2", target_bir_lowering=False, debug=False,
                   num_devices=NCORES)

    eaT_d = nc.dram_tensor("eaT", [64, Ep], F32, kind="ExternalInput").ap()
    xs_d = nc.dram_tensor("xs", [Ep, 64], F32, kind="ExternalInput").ap()
    sh_d = nc.dram_tensor("sh", [Ep, 4], F32, kind="ExternalInput").ap()
    dst_d = nc.dram_tensor("dstf", [Ep], F32, kind="ExternalInput").ap()
    w1_d = nc.dram_tensor("w1", [64, 64], F32, kind="ExternalInput").ap()
    b1_d = nc.dram_tensor("b1", [64, 1], F32, kind="ExternalInput").ap()
    w2_d = nc.dram_tensor("w2p", [64, 1024], BF16, kind="ExternalInput").ap()
    rc_d = nc.dram_tensor("recipT", [128, GROUPS], F32, kind="ExternalInput").ap()
    io_d = nc.dram_tensor("iota", [128, 128], F32, kind="ExternalInput").ap()
    if b2_nonzero:
        b2_d = nc.dram_tensor("b2bc", [128, 1024], BF16, kind="ExternalInput").ap()
    out_d = nc.dram_tensor("out", [NPC, 64], F32, kind="ExternalOutput").ap()

    with tile.TileContext(nc) as tc:
        with (
            tc.tile_pool(name="singles", bufs=1) as singles,
            tc.tile_pool(name="sup", bufs=3) as sup,
            tc.tile_pool(name="chunk", bufs=3) as chk,
            tc.tile_pool(name="post", bufs=2) as post,
            tc.tile_pool(name="psT", bufs=2, space="PSUM") as psT,
            tc.tile_pool(name="psN", bufs=1, space="PSUM") as psN,
        ):
            w1sb = singles.tile([64, 64], F32)
            b1sb = singles.tile([64, 1], F32)
            w2sb = singles.tile([64, 1024], BF16)
            rcsb = singles.tile([128, GROUPS], F32)
            iosb = singles.tile([128, 128], F32)
            nc.sync.dma_start(w1sb, w1_d)
            nc.sync.dma_start(b1sb, b1_d)
            nc.sync.dma_start(w2sb, w2_d)
            nc.sync.dma_start(rcsb, rc_d)
            nc.sync.dma_start(iosb, io_d)
            if b2_nonzero:
                b2sb = singles.tile([128, 1024], BF16)
                nc.sync.dma_start(b2sb, b2_d)

            sup_state = {}

            def do_supertile(s):
                eat = sup.tile([64, 512], F32, tag="ea")
                nc.sync.dma_start(eat, _dram_ap(eaT_d, s * 512, [[Ep, 64], [1, 512]]))
                xst = sup.tile([128, 4, 64], F32, tag="x")
                nc.sync.dma_start(
                    xst, _dram_ap(xs_d, s * 512 * 64, [[64, 128], [128 * 64, 4], [1, 64]]))
                sht = sup.tile([128, 4, 4], F32, tag="sh")
                nc.sync.dma_start(
                    sht, _dram_ap(sh_d, s * 512 * 4, [[4, 128], [128 * 4, 4], [1, 4]]))
                dstt = sup.tile([128, 4], F32, tag="dst")
                nc.sync.dma_start(
                    dstt, _dram_ap(dst_d, s * 512, [[1, 128], [128, 4]]))
                hps = psT.tile([64, 512], F32, tag="T")
                nc.tensor.matmul(hps[:], w1sb[:], eat[:], start=True, stop=True)
                hb = sup.tile([64, 512], BF16, tag="hb")
                nc.scalar.activation(hb[:], hps[:],
                                     mybir.ActivationFunctionType.Silu,
                                     bias=b1sb[:, 0:1], scale=1.0)
                sup_state.update(xst=xst, sht=sht, dstt=dstt, hb=hb)

            for g in range(GROUPS):
                npsum = psN.tile([128, 2048], F32, tag="N")
                for k in range(K):
                    i = g * K + k
                    s, q = divmod(i, 4)
                    if q == 0:
                        do_supertile(s)
                    xst, sht, dstt, hb = (sup_state["xst"], sup_state["sht"],
                                          sup_state["dstt"], sup_state["hb"])

                    # T = hb_chunk.T @ w2perm  -> [128e, 1024] psum
                    Tps = psT.tile([128, 1024], F32, tag="T")
                    nc.tensor.matmul(Tps[:, 0:512], hb[:, q * 128:(q + 1) * 128],
                                     w2sb[:, 0:512], start=True, stop=True)
                    nc.tensor.matmul(Tps[:, 512:1024], hb[:, q * 128:(q + 1) * 128],
                                     w2sb[:, 512:1024], start=True, stop=True)
                    # evict to bf16 SBUF (split ACT/DVE)
                    Tsb = chk.tile([128, 1024], BF16, tag="Tsb")
                    nc.scalar.copy(Tsb[:, 0:512], Tps[:, 0:512])
                    nc.vector.tensor_copy(Tsb[:, 512:1024], Tps[:, 512:1024])
                    if b2_nonzero:
                        nc.vector.tensor_add(Tsb[:], Tsb[:], b2sb[:])

                    # ---- v-build: vb[128,128] = [a0 | y'(m,u) | v01(m,u) | a1]
                    vb = chk.tile([128, 128], BF16, tag="vb")
                    xo = q * 64
                    so = q * 4
                    sh0 = _mkap(sht, so, [[1, 1]])
                    # a0 = x0*sh0
                    nc.vector.tensor_scalar_mul(
                        vb[:, 0:16], _mkap(xst, xo, [[1, 16]]), sh0)
                    # y'[m,u] = x1[u,m]*sh0   (x1[u,m] at col 16+3u+m)
                    nc.vector.tensor_scalar_mul(
                        _mkap(vb, 16, [[16, 3], [1, 16]]),
                        _mkap(xst, xo + 16, [[1, 3], [3, 16]]), sh0)
                    # v01[m,u] = x0[u]*sh1[m]
                    nc.vector.tensor_tensor(
                        out=_mkap(vb, 64, [[16, 3], [1, 16]]),
                        in0=_mkap(xst, xo, [[0, 3], [1, 16]]),
                        in1=_mkap(sht, so + 1, [[1, 3], [0, 16]]),
                        op=mybir.AluOpType.mult)
                    # a1[u] = sum_m x1[u,m]*sh1[m]
                    tmp = chk.tile([128, 48], F32, tag="tmp")
                    nc.vector.tensor_tensor(
                        out=_mkap(tmp, 0, [[3, 16], [1, 3]]),
                        in0=_mkap(xst, xo + 16, [[3, 16], [1, 3]]),
                        in1=_mkap(sht, so + 1, [[0, 16], [1, 3]]),
                        op=mybir.AluOpType.mult)
                    a1f = chk.tile([128, 16], F32, tag="a1f")
                    nc.vector.tensor_reduce(
                        out=a1f[:], in_=_mkap(tmp, 0, [[3, 16], [1, 3]]),
                        axis=mybir.AxisListType.X, op=mybir.AluOpType.add)
                    nc.vector.tensor_copy(vb[:, 112:128], a1f[:])

                    # ---- multiplies: t[128, 2048] bf16
                    t = chk.tile([128, 2048], BF16, tag="t")
                    # A: (t00|t11) = T[j,o,u]*v[pair_j, u], pairs at cols 0 / 112
                    nc.vector.tensor_tensor(
                        out=_mkap(t, 0, [[256, 2], [16, 16], [1, 16]]),
                        in0=_mkap(Tsb, 0, [[256, 2], [16, 16], [1, 16]]),
                        in1=_mkap(vb, 0, [[112, 2], [0, 16], [1, 16]]),
                        op=mybir.AluOpType.mult)
                    # B: t01_m = T01[o,u]*v01[m,u]
                    nc.vector.tensor_tensor(
                        out=_mkap(t, 512, [[256, 3], [16, 16], [1, 16]]),
                        in0=_mkap(Tsb, 512, [[0, 3], [16, 16], [1, 16]]),
                        in1=_mkap(vb, 64, [[16, 3], [0, 16], [1, 16]]),
                        op=mybir.AluOpType.mult)
                    # C: t10_m = T10[o,u]*y'[m,u]
                    nc.vector.tensor_tensor(
                        out=_mkap(t, 1280, [[256, 3], [16, 16], [1, 16]]),
                        in0=_mkap(Tsb, 768, [[0, 3], [16, 16], [1, 16]]),
                        in1=_mkap(vb, 16, [[16, 3], [0, 16], [1, 16]]),
                        op=mybir.AluOpType.mult)

                    # ---- B one-hot on gpsimd + scatter matmuls
                    Bt = chk.tile([128, 128], BF16, tag="Bt")
                    nc.gpsimd.tensor_scalar(out=Bt[:], in0=iosb[:],
                                            scalar1=_mkap(dstt, q, [[1, 1]]),
                                            scalar2=None,
                                            op0=mybir.AluOpType.is_equal)
                    for j in range(4):
                        nc.tensor.matmul(npsum[:, j * 512:(j + 1) * 512], Bt[:],
                                         t[:, j * 512:(j + 1) * 512],
                                         start=(k == 0), stop=(k == K - 1))

                # ---- group post-process
                r = post.tile([128, 128], F32, tag="r")
                nc.vector.tensor_reduce(
                    out=r[:], in_=_mkap(npsum, 0, [[16, 128], [1, 16]]),
                    axis=mybir.AxisListType.X, op=mybir.AluOpType.add)
                Fc = post.tile([128, 64], F32, tag="F")
                nc.vector.tensor_add(Fc[:, 0:16], r[:, 0:16], r[:, 16:32])
                nc.vector.tensor_tensor(
                    out=_mkap(Fc, 16, [[3, 16], [1, 3]]),
                    in0=_mkap(r, 32, [[1, 16], [16, 3]]),
                    in1=_mkap(r, 80, [[1, 16], [16, 3]]),
                    op=mybir.AluOpType.add)
                nc.vector.tensor_scalar_mul(Fc[:], Fc[:], rcsb[:, g:g + 1])
                nc.sync.dma_start(
                    _dram_ap(out_d, g * 128 * 64, [[64, 128], [1, 64]]), Fc[:])

    nc.compile()
    return nc


def _prep_inputs(node_attr, edge_index, edge_attr, edge_sh, w1, b1, w2, b2):
    src, dst = edge_index[0].astype(np.int64), edge_index[1].astype(np.int64)

    node_deg = np.bincount(dst, minlength=N)
    group_sizes = node_deg.reshape(NCORES * GROUPS, 128).sum(1)
    K = max(1, int(np.ceil(group_sizes.max() / 128)))
    C = GROUPS * K           # GROUPS=32 -> C always divisible by 4
    Ep = C * 128

    # permuted & scaled w2: col p*256 + o*16 + u, path order (00, 11, 01, 10)
    w2r = w2.reshape(64, 4, MUL, MUL)  # [c, path, u, o]
    scale = np.array([ALPHA, ALPHA / SQRT3, ALPHA, ALPHA], dtype=np.float32)
    path_order = [0, 3, 1, 2]
    w2p = np.stack([w2r[:, p].transpose(0, 2, 1) * scale[pi]
                    for pi, p in enumerate(path_order)], axis=1)  # [c, 4, o, u]
    w2p = np.ascontiguousarray(w2p.reshape(64, 1024)).astype(BF16NP)

    b2_nonzero = bool(np.any(b2))
    b2bc = None
    if b2_nonzero:
        b2r = b2.reshape(4, MUL, MUL)
        b2p = np.stack([b2r[p].T * scale[pi]
                        for pi, p in enumerate(path_order)], axis=0).reshape(1024)
        b2bc = np.tile(b2p.astype(BF16NP)[None, :], (128, 1))

    iota = np.tile(np.arange(128, dtype=np.float32), (128, 1))

    per_core = []
    for c in range(NCORES):
        base = c * NPC
        sel = np.nonzero((dst >= base) & (dst < base + NPC))[0]
        d_loc = dst[sel] - base
        order = np.argsort(d_loc, kind="stable")
        sel = sel[order]
        d_loc = d_loc[order]
        g = d_loc >> 7
        cnt_g = np.bincount(g, minlength=GROUPS)
        assert cnt_g.max() <= K * 128, "K underestimated"
        group_off = np.arange(GROUPS, dtype=np.int64) * (K * 128)
        starts = np.cumsum(cnt_g) - cnt_g
        within = np.arange(len(sel), dtype=np.int64) - np.repeat(starts, cnt_g)
        pos = group_off[g] + within

        ea_p = np.zeros((Ep, 64), np.float32)
        ea_p[pos] = edge_attr[sel]
        x_p = np.zeros((Ep, 64), np.float32)
        x_p[pos] = node_attr[src[sel]]
        sh_p = np.zeros((Ep, 4), np.float32)
        sh_p[pos] = edge_sh[sel]
        dstf = np.full(Ep, -1.0, np.float32)
        dstf[pos] = (d_loc & 127).astype(np.float32)
        deg = node_deg[base:base + NPC]
        recipT = np.ascontiguousarray(
            (1.0 / np.maximum(deg, 1)).astype(np.float32).reshape(GROUPS, 128).T)
        in_map = {
            "eaT": np.ascontiguousarray(ea_p.T),
            "xs": x_p,
            "sh": sh_p,
            "dstf": dstf,
            "w1": w1.astype(np.float32),
            "b1": b1.reshape(64, 1).astype(np.float32),
            "w2p": w2p,
            "recipT": recipT,
            "iota": iota,
        }
        if b2_nonzero:
            in_map["b2bc"] = b2bc
        per_core.append(in_map)
    return K, b2_nonzero, per_core


class _Runner:
    """PJRT SPMD executor for a compiled Bass program (mirrors
    concourse.bass2jax.run_bass_via_pjrt, but caches the jitted callable so
    repeat calls / benchmarking don't recompile)."""

    def __init__(self, nc):
        import jax
        from jax.sharding import Mesh, PartitionSpec
        from jax.experimental.shard_map import shard_map
        from concourse import bass2jax as b2j

        b2j.install_neuronx_cc_hook()
        self.nc = nc
        partition_name = (nc.partition_id_tensor.name
                          if nc.partition_id_tensor else None)
        in_names, out_names, out_avals, zero_outs = [], [], [], []
        for alloc in nc.m.functions[0].allocations:
            if not isinstance(alloc, mybir.MemoryLocationSet):
                continue
            name = alloc.memorylocations[0].name
            if alloc.kind == "ExternalInput":
                if name != partition_name:
                    in_names.append(name)
            elif alloc.kind == "ExternalOutput":
                out_names.append(name)
                shape = tuple(alloc.tensor_shape)
                dtype = mybir.dt.np(alloc.dtype)
                out_avals.append(jax.core.ShapedArray(shape, dtype))
                zero_outs.append(np.zeros(shape, dtype))
        self.in_names, self.out_names = in_names, out_names
        self.out_avals, self.zero_outs = out_avals, zero_outs
        n_params, n_outs = len(in_names), len(out_names)
        all_names = in_names + out_names
        if partition_name is not None:
            all_names = all_names + [partition_name]
        all_names = tuple(all_names)

        def _body(*args):
            operands = list(args)
            if partition_name is not None:
                operands.append(b2j.partition_id_tensor())
            outs = b2j._bass_exec_p.bind(
                *operands,
                out_avals=tuple(out_avals),
                in_names=all_names,
                out_names=tuple(out_names),
                lowering_input_output_aliases=(),
                sim_require_finite=True,
                sim_require_nnan=True,
                nc=nc,
            )
            return tuple(outs)

        devices = jax.devices()[:NCORES]
        self.mesh = Mesh(np.asarray(devices), ("core",))
        in_specs = (PartitionSpec("core"),) * (n_params + n_outs)
        out_specs = (PartitionSpec("core"),) * n_outs
        # kernel writes every output element -> no donation needed; this lets
        # us reuse device-resident zero buffers across benchmark repeats.
        self.fn = jax.jit(
            shard_map(_body, mesh=self.mesh, in_specs=in_specs,
                      out_specs=out_specs, check_rep=False),
            keep_unused=True,
        )
        self._jax = jax
        self._dev_args = None

    def put(self, per_core):
        """Transfer per-core input dicts to devices (concat on axis 0)."""
        jax = self._jax
        from jax.sharding import NamedSharding, PartitionSpec
        sh = NamedSharding(self.mesh, PartitionSpec("core"))
        args = []
        for name in self.in_names:
            cat = np.concatenate([np.asarray(m[name]) for m in per_core], axis=0)
            args.append(jax.device_put(cat, sh))
        for z in self.zero_outs:
            cat = np.zeros((NCORES * z.shape[0], *z.shape[1:]), z.dtype)
            args.append(jax.device_put(cat, sh))
        self._dev_args = args

    def run(self):
        outs = self.fn(*self._dev_args)
        return [np.asarray(o) for o in outs]

    def bench(self, n=20):
        import time
        self.run()[0].sum()  # warm
        times = []
        for _ in range(n):
            t0 = time.perf_counter()
            outs = self.fn(*self._dev_args)
            self._jax.block_until_ready(outs)
            times.append(time.perf_counter() - t0)
        return times


def _get_runner(K, b2_nonzero):
    key = (K, b2_nonzero)
    if key not in _PROGRAM_CACHE:
        nc = _build_program(K, b2_nonzero)
        _PROGRAM_CACHE[key] = _Runner(nc)
    return _PROGRAM_CACHE[key]


def kernel(node_attr, edge_index, edge_attr, edge_sh, w1, b1, w2, b2):
    global LAST_RESULT
    node_attr = np.asarray(node_attr, np.float32)
    edge_index = np.asarray(edge_index)
    edge_attr = np.asarray(edge_attr, np.float32)
    edge_sh = np.asarray(edge_sh, np.float32)
    w1 = np.asarray(w1, np.float32)
    b1 = np.asarray(b1, np.float32)
    w2 = np.asarray(w2, np.float32)
    b2 = np.asarray(b2, np.float32)

    K, b2_nonzero, per_core = _prep_inputs(
        node_attr, edge_index, edge_attr, edge_sh, w1, b1, w2, b2)
    runner = _get_runner(K, b2_nonzero)
    runner.put(per_core)
    outs = runner.run()
    LAST_RESULT = runner
    oi = runner.out_names.index("out")
    full = outs[oi].reshape(NCORES, NPC, 64).reshape(N, 64)
    return full.astype(np.float32)
